# revision 9
# baseline (speedup 1.0000x reference)
"""Trainium2 Bass kernel for MultiHeadSelfAttention + residual + LayerNorm.

Problem: x[2, 2048, 1024], 16 heads, head_dim 64, fp32 I/O.
  Q/K/V = x @ W{q,k,v}.T + b;  attn = softmax(Q K^T / 8) V
  out = attn-concat @ Wo.T + bo;  y = LayerNorm(x + out)

Sharding (8 cores, collective-free):
  core c: batch b = c // 4, query-token strip q = c % 4 (512 tokens).
  Each core computes K/V for its whole batch (all 16 heads), Q for its
  512 query tokens, full attention + out-proj + LayerNorm for them, and
  outputs out[512, 1024].  K/V projection is recomputed 4x per batch --
  cheaper than the measured collective alternatives for this shape.

v2 design (PE was the bottleneck at ~259us busy of a 310us span):
  - ALL matmuls run fp8 e4m3 (x, Wq, Wk, Wv, Wo, K, Q, P, V, ctx in fp8)
    with DoubleRow pair-contraction for the projections and out-proj:
    halves the MM count of K/Q/out projections vs the bf16 baseline.
  - Weights are pre-scaled host-side to sit in the e4m3 normal range
    (Wq,Wk x8; Wv x16; Wo x32) and the residual input is pre-scaled x512
    so the out-proj PSUM lands at 512*(out+x).  LayerNorm is
    scale-invariant, so only eps is scaled (x512^2); gamma/beta epilogue
    is unaffected.  Score scale absorbs the 8*8: exp(scale=0.125/64).
  - Scores (contraction = head_dim 64 -> only half the PE rows) run as
    row-tiled CONCURRENT pairs: head-even weights in array rows 0-63,
    head-odd in rows 64-127 (tile_position auto-derived from the base
    partition), sharing the 128x128 array per key chunk.
  - x is loaded once (fp8, 2MB) and stays resident; the bf16 copy of x
    is gone entirely (halves input DMA).
  - softmax exp: most chunks on the Scalar engine (Exp LUT, fp8 out);
    every 4th chunk is computed on the Vector engine instead with a
    Schraudolph bit-trick: bits = round(a*logits + 55.54) as uint8,
    bitcast to e4m3 (DVE converts with round-to-nearest, saturating).
    Zero-bias constant so ACT and DVE chunks agree in expectation;
    softmax renormalizes the shared multiplicative bias away anyway
    (the ones-column in V gives rowsums of the SAME quantized P).
  - rowsum reciprocal runs directly on the PSUM rowsum row ([1,512]),
    then gpsimd partition-broadcasts the reciprocal (drops one DVE
    copy per head vs broadcasting the raw sum).
  - Only K/Q j-tiles 0-1 run before the attention loop; V quads 0..3 and
    the remaining j-tiles stream in as PE filler inside the attention
    windows (scheduled so each pair's K/Q/V inputs land one pair ahead),
    so the first exp fires ~15us in, the PE never idles long, and the
    HAM clock stays warm.
  - rowsum reciprocal: DVE copy of the PSUM rowsum row -> fp32
    reciprocal_approx_fast -> gpsimd partition-broadcast.  (The custom
    DVE reciprocal silently corrupts when its input AP has a nonzero
    base partition or lives in PSUM -- keep it fed from an SBUF tile at
    partition 0.)
  - The upfront j0/j1 K/Q evacuations run on the Scalar engine (Identity
    + per-partition bias) in its pre-exp idle window; out-proj PSUM for
    both halves comes from the pmm pool (idle after the last proj
    filler) so its j<3 accumulation MMs hoist under pairs 6-7 instead of
    waiting for the final exp to free an smm bank.  wk8 is DMA'd in
    column halves (j0-3 first) and the last pair's final chunk runs on
    the DVE so the closing PV never queues behind ACT's FIFO.
  - PV emission is delayed one chunk-pair: the PE executes matmuls in
    strict FIFO order, so a PV waiting on its exp output would block the
    next chunk's (independent, already-ready) score matmuls; delaying PV
    one iteration fills that bubble with the following scores.
  - V projection runs as token-chunk x 8-head (N=512) matmul groups:
    half the matmul count and half the DVE evacuations of the older
    4-head/N=256 quads.
  - PSUM (8 banks) is the binding constraint on further restructuring:
    score double-buffering (4) + two ctx accumulators (2) + proj-filler
    double-buffering (2) is exactly 8, which rules out 4-head row-tiled
    scores and 2048-wide exp batches (both need >=10 banks).
Measured: HW exec ~228-230us (baseline bf16 kernel: ~300us); end-to-end
Frobenius rel err ~6.7e-4 (tolerance 2e-2; errors in the attention path
are suppressed ~100x by the residual, so fp8 everywhere is safe).
Note: cross-core AllGather K/V sharing was implemented and was
numerically correct, but each 0.5MB collective costs ~20us serial
latency under this runtime and two collectives serialize on gpsimd --
it measured 323us and was reverted.
"""

import numpy as np
import ml_dtypes

P = 128
D = 1024
S = 2048
B = 2
H = 16
DH = 64
TQ = 512  # query tokens per core
N_CORES = 8

F32 = np.float32
BF16 = ml_dtypes.bfloat16
F8 = ml_dtypes.float8_e4m3fn

# host-side pre-scales (see docstring)
WQ_S = 8.0
WK_S = 8.0
WV_S = 16.0
WO_S = 32.0
RES_S = WV_S * WO_S  # 512
LN_EPS = 1e-5 * RES_S * RES_S
SC_SCALE = 0.125 / (WQ_S * WK_S)  # exp scale on raw psum logits
A_SCHR = SC_SCALE * 8.0 / 0.6931471805599453
B_SCHR = 55.54
# chunk indices (of 16 per head-pair) done on DVE instead of ACT.
# Pairs 0-3 are PE-bound (projection fillers) so ACT takes most chunks;
# pairs 4-7 have light/no fillers and are ACT-exp-bound, so more chunks
# shift to the DVE (which has no evacuation work left by then).
SCHR_BY_PAIR = {
    0: (3, 8, 13),
    1: (3, 8, 13),
    2: (3, 8, 13),
    3: (3, 8, 13),
    4: (3, 6, 9, 12, 15),
    5: (3, 6, 9, 12, 15),
    6: (2, 5, 8, 11, 13, 15),
    7: (2, 5, 8, 11, 13, 15),
}

_BUILT = {}

import os

KPHASE = int(os.environ.get("KPHASE", "3"))
KDEBUG = int(os.environ.get("KDEBUG", "0"))


def _build_nc(apply_gb=True):
    from contextlib import ExitStack

    import concourse.tile as tile
    from concourse import bacc, mybir

    bf = mybir.dt.bfloat16
    f8 = mybir.dt.float8e4
    u8 = mybir.dt.uint8
    f32 = mybir.dt.float32
    AX = mybir.AxisListType.X
    OP = mybir.AluOpType
    AF = mybir.ActivationFunctionType
    DR = mybir.MatmulPerfMode.DoubleRow

    nc = bacc.Bacc(
        "TRN2",
        target_bir_lowering=False,
        debug=False,
        enable_asserts=False,
        num_devices=N_CORES,
    )

    # ---- DRAM I/O ----
    # x8 is strip-major [strip, p, k, 512] so the first K-proj group can
    # start after 1MB of DMA (x strip0 + wk half0) instead of 2.5MB
    x8_d = nc.dram_tensor("x8", [4, P, 8, 512], f8, kind="ExternalInput").ap()
    wq_d = nc.dram_tensor("wq", [D, D], f8, kind="ExternalInput").ap()
    wk_d = nc.dram_tensor("wk", [D, D], f8, kind="ExternalInput").ap()
    wv_d = nc.dram_tensor("wv", [D, D], f8, kind="ExternalInput").ap()
    wo_d = nc.dram_tensor("wo", [D, D], f8, kind="ExternalInput").ap()
    qb_d = nc.dram_tensor("qb", [P, 8], f32, kind="ExternalInput").ap()
    kb_d = nc.dram_tensor("kb", [P, 8], f32, kind="ExternalInput").ap()
    # rows: [16*bv | gamma | beta]
    rows_d = nc.dram_tensor("rows", [1, 3 * D], bf, kind="ExternalInput").ap()
    xres_d = nc.dram_tensor("xres", [TQ, D], f32, kind="ExternalInput").ap()
    out_d = nc.dram_tensor("out", [TQ, D], f32, kind="ExternalOutput").ap()
    if KDEBUG:
        dbg_k = nc.dram_tensor("dbg_k", [P, 8 * S], u8, kind="ExternalOutput").ap()
        dbg_q = nc.dram_tensor("dbg_q", [P, 8 * TQ], u8, kind="ExternalOutput").ap()
        dbg_v = nc.dram_tensor("dbg_v", [P, 16 * H * (DH + 1)], u8, kind="ExternalOutput").ap()
        dbg_c = nc.dram_tensor("dbg_c", [P, 8 * TQ], u8, kind="ExternalOutput").ap()
        dbg_pt = nc.dram_tensor("dbg_pt", [P, 8, 2, 2, 512], u8, kind="ExternalOutput").ap()
        dbg_cp = nc.dram_tensor("dbg_cp", [P, 2, 512], f32, kind="ExternalOutput").ap()

    wq_t = wq_d.rearrange("(o p) n -> p o n", p=P)  # [128, 8, 1024]
    wk_t = wk_d.rearrange("(o p) n -> p o n", p=P)
    wv_t = wv_d.rearrange("(o p) n -> p o n", p=P)
    wo_t = wo_d.rearrange("(o p) n -> p o n", p=P)

    with tile.TileContext(nc) as tc:
        with ExitStack() as ctx:
            # ---- pools ----
            consts = ctx.enter_context(tc.tile_pool(name="consts", bufs=1))
            wpool = ctx.enter_context(tc.tile_pool(name="wpool", bufs=1))
            big = ctx.enter_context(tc.tile_pool(name="big", bufs=1))
            ppool = ctx.enter_context(tc.tile_pool(name="ppool", bufs=3))
            spool = ctx.enter_context(tc.tile_pool(name="spool", bufs=4))
            hpool = ctx.enter_context(tc.tile_pool(name="hpool", bufs=3))
            xrpool = ctx.enter_context(tc.tile_pool(name="xrpool", bufs=4))
            pmm = ctx.enter_context(tc.tile_pool(name="pmm", bufs=2, space="PSUM"))
            smm = ctx.enter_context(tc.tile_pool(name="smm", bufs=2, space="PSUM"))
            ctxp = ctx.enter_context(tc.tile_pool(name="ctxp", bufs=2, space="PSUM"))

            # ---- constants ----
            zero_c = consts.tile([P, 1], f32, tag="zero_c")
            nc.vector.memset(zero_c[:], 0.0)
            nc.const_aps.aps[(f32, 0.0)] = zero_c[:]
            eps_c = consts.tile([P, 1], f32, tag="eps_c")
            nc.vector.memset(eps_c[:], LN_EPS)
            ones_l = consts.tile([1, P], bf, tag="ones_l")  # matmul lhsT ones
            nc.vector.memset(ones_l[:], 1.0)
            rows_sb = consts.tile([1, 3 * D], bf, tag="rows")
            nc.sync.dma_start(rows_sb[:], rows_d[:])
            qb_sb = consts.tile([P, 8], f32, tag="qb")
            nc.sync.dma_start(qb_sb[:], qb_d[:])
            kb_sb = consts.tile([P, 8], f32, tag="kb")
            nc.sync.dma_start(kb_sb[:], kb_d[:])

            # ---- resident inputs ----
            x8_sb = wpool.tile([P, 8, S], f8, tag="x8")
            wk8_sb = wpool.tile([P, 8, D], f8, tag="wk8")
            wv8_sb = wpool.tile([P, 8, D], f8, tag="wv8")
            wq8_sb = wpool.tile([P, 8, D], f8, tag="wq8")
            wo8_sb = wpool.tile([P, 8, D], f8, tag="wo8")
            # DMA order matched to compute order: the upfront K(j0/j1)+Q
            # projections need only x-strip0..3 + wk/wq column-half0, so
            # those 3MB land first; wv half0 before the later halves (the
            # V-proj fillers of pair 0 consume it from ~14us).
            for k in range(8):
                nc.sync.dma_start(x8_sb[:, k, 0:512], x8_d[0, :, k])
            for k in range(8):
                nc.sync.dma_start(wk8_sb[:, k, :512], wk_t[:, k, :512])
            for k in range(8):
                nc.sync.dma_start(x8_sb[:, k, 512:1024], x8_d[1, :, k])
            for k in range(8):
                nc.sync.dma_start(wq8_sb[:, k, :512], wq_t[:, k, :512])
            for k in range(8):
                nc.sync.dma_start(x8_sb[:, k, 1024:1536], x8_d[2, :, k])
            for k in range(8):
                nc.sync.dma_start(x8_sb[:, k, 1536:2048], x8_d[3, :, k])
            for k in range(8):
                nc.sync.dma_start(wv8_sb[:, k, :512], wv_t[:, k, :512])
            for k in range(8):
                nc.sync.dma_start(wv8_sb[:, k, 512:], wv_t[:, k, 512:])
            for k in range(8):
                nc.sync.dma_start(wk8_sb[:, k, 512:], wk_t[:, k, 512:])
            for k in range(8):
                nc.sync.dma_start(wq8_sb[:, k, 512:], wq_t[:, k, 512:])

            # broadcast [1, 1024] rows across partitions via rank-1 matmuls
            bv_bc = consts.tile([P, D], bf, tag="bv_bc")
            bcasts = [bv_bc]
            if apply_gb:
                ga_bc = consts.tile([P, D], bf, tag="ga_bc")
                be_bc = consts.tile([P, D], bf, tag="be_bc")
                bcasts += [ga_bc, be_bc]
            for idx, dst in enumerate(bcasts):
                for half in range(2):
                    ps = smm.tile([P, 2, 512], f32, tag="smm")
                    nc.tensor.matmul(
                        ps[:, 0],
                        ones_l[:],
                        rows_sb[:, idx * D + half * 512 : idx * D + (half + 1) * 512],
                        start=True,
                        stop=True,
                    )
                    nc.scalar.copy(dst[:, half * 512 : (half + 1) * 512], ps[:, 0])

            # ---- big activations ----
            kT8 = big.tile([P, 8, S], f8, tag="kT")  # K^T: [dh-pair part, j, token]
            qT8 = big.tile([P, 8, TQ], f8, tag="qT")
            # V' per (tk-chunk, head): [128 tok, 65] (64 dh + ones col)
            v_sb = big.tile([P, 16, H, DH + 1], f8, tag="v")
            nc.vector.memset(v_sb[:, :, :, DH : DH + 1], 1.0)
            ctxf = [
                big.tile([P, 2, TQ], f8, tag=f"ctxf{q}", name=f"ctxf{q}")
                for q in range(4)
            ]

            # ---- fp8 DoubleRow K/Q projection for one j-tile (all strips) ----
            # evac_act=True moves the bias-add evacuation to the Scalar
            # engine (Identity + per-partition bias) -- used for the
            # upfront j-tiles, which land in ACT's pre-exp idle window
            def kq_proj_j(j, evac_act=False):
                def evac(dst, ps, bias):
                    if evac_act:
                        nc.scalar.activation(dst, ps, AF.Identity, bias=bias)
                    else:
                        nc.vector.tensor_scalar_add(dst, ps, bias)

                steps = []
                for s in range(4):
                    def kstep(s=s, j=j):
                        ps = pmm.tile([P, 512], f32, tag="pmm")
                        for c2 in range(4):
                            nc.tensor.matmul(
                                ps[:],
                                wk8_sb[:, 2 * c2 : 2 * c2 + 2, j * P : (j + 1) * P],
                                x8_sb[:, 2 * c2 : 2 * c2 + 2, s * 512 : (s + 1) * 512],
                                start=(c2 == 0),
                                stop=(c2 == 3),
                                perf_mode=DR,
                            )
                        evac(
                            kT8[:, j, s * 512 : (s + 1) * 512], ps[:], kb_sb[:, j : j + 1]
                        )
                    steps.append(kstep)

                def qstep(j=j):
                    ps = pmm.tile([P, 512], f32, tag="pmm")
                    for c2 in range(4):
                        nc.tensor.matmul(
                            ps[:],
                            wq8_sb[:, 2 * c2 : 2 * c2 + 2, j * P : (j + 1) * P],
                            x8_sb[:, 2 * c2 : 2 * c2 + 2, 0:512],
                            start=(c2 == 0),
                            stop=(c2 == 3),
                            perf_mode=DR,
                        )
                    evac(qT8[:, j], ps[:], qb_sb[:, j : j + 1])
                steps.append(qstep)
                return steps

            # ---- fp8 DoubleRow V projection: one token chunk x 8 heads
            # (N=512) per step -- half the MM and evacuation count of the
            # old 4-head/N=256 quads ----
            def v_proj_half(half):
                steps = []
                for tchunk in range(16):
                    def vstep(tchunk=tchunk, half=half):
                        ps = pmm.tile([P, 512], f32, tag="pmm")
                        for c2 in range(4):
                            nc.tensor.matmul(
                                ps[:],
                                x8_sb[:, 2 * c2 : 2 * c2 + 2, tchunk * P : (tchunk + 1) * P],
                                wv8_sb[:, 2 * c2 : 2 * c2 + 2, half * 512 : (half + 1) * 512],
                                start=(c2 == 0),
                                stop=(c2 == 3),
                                perf_mode=DR,
                            )
                        nc.vector.tensor_tensor(
                            v_sb[:, tchunk, half * 8 : (half + 1) * 8, 0:DH],
                            ps[:].rearrange("p (h d) -> p h d", d=DH),
                            bv_bc[:, half * 512 : (half + 1) * 512].rearrange(
                                "p (h d) -> p h d", d=DH
                            ),
                            OP.add,
                        )
                    steps.append(vstep)
                return steps

            # upfront: j0, j1 K/Q projections only; V half 0 streams in as
            # pair-0 filler so the first exp can start ~15us earlier.
            # All K groups are emitted before the Q steps: Q's matmuls wait
            # on the later wq8 DMA, and emitting them interleaved would
            # block the (ready) K(j1) groups behind them in the PE FIFO.
            up0 = kq_proj_j(0, evac_act=True)
            up1 = kq_proj_j(1, evac_act=True)
            for st in up0[:4]:
                st()
            for st in up1[:4]:
                st()
            up0[4]()
            up1[4]()

            # filler schedule per attention pair (consumed one per chunk)
            def dma_fill():
                for k in range(8):
                    nc.sync.dma_start(wo8_sb[:, k], wo_t[:, k])

            xrs = []

            def xres_fill():
                for tt in range(4):
                    xr = xrpool.tile([P, D], f32, tag="xr", name=f"xr{tt}")
                    nc.sync.dma_start(xr[:], xres_d[tt * P : (tt + 1) * P, :])
                    xrs.append(xr)

            vh = [v_proj_half(h) for h in range(2)]

            # ---- out-projection j0-2 accumulation groups ----
            # Emitted early (pair-6 fillers + right after pair 7's last PV)
            # so they execute during the otherwise PE-idle exp/normalize
            # windows of the last two pairs; all 8 PSUM banks carry an open
            # out-proj group at the tail.  j3 (which needs the final pair's
            # ctx) lands in the same banks via start=False accumulation.
            op_ps = {}

            def op_j02(tt, half, ps):
                op_ps[(tt, half)] = ps
                for j in range(3):
                    nc.tensor.matmul(
                        ps,
                        ctxf[j][:, :, tt * P : (tt + 1) * P],
                        wo8_sb[:, 2 * j : 2 * j + 2, half * 512 : (half + 1) * 512],
                        start=(j == 0),
                        stop=(j == 2),
                        perf_mode=DR,
                    )

            def op_fill(tt, half):
                def step():
                    ps = pmm.tile([P, 512], f32, tag="pmm")
                    op_j02(tt, half, ps[:])
                return step

            # wo8/xres prefetch sits at pair 3, NOT the tail: the out-proj
            # j0-2 groups can only hoist under pairs 6-7 if wo8 has landed
            fillers = [
                vh[0],
                kq_proj_j(2) + kq_proj_j(3) + vh[1][:6],
                vh[1][6:] + kq_proj_j(4),
                kq_proj_j(5) + [dma_fill, xres_fill],
                kq_proj_j(6),
                kq_proj_j(7),
                [op_fill(0, 0), op_fill(1, 0)] if KPHASE >= 3 else [],
                [],
            ]

            # ---- attention: 8 head-pairs, row-tiled concurrent score MMs ----
            for pr in range(8):
                he, ho = 2 * pr, 2 * pr + 1
                # pairs 4-7 end on a DVE chunk so the closing PV+normalize
                # chain never queues behind ACT's FIFO
                schr = SCHR_BY_PAIR[pr]
                fill = list(fillers[pr])
                if KPHASE < 2:
                    for st in fill:
                        st()
                    continue
                cps_e = ctxp.tile([P, 512], f32, tag="ctx")
                cps_o = ctxp.tile([P, 512], f32, tag="ctx")

                # PV for chunk-pair idx (both heads)
                def pv(idx, pt_):
                    nc.tensor.matmul(
                        cps_e[: DH + 1],
                        v_sb[:, 2 * idx : 2 * idx + 2, he],
                        pt_[:, :, 0],
                        start=(idx == 0),
                        stop=(idx == 7),
                        perf_mode=DR,
                    )
                    nc.tensor.matmul(
                        cps_o[: DH + 1],
                        v_sb[:, 2 * idx : 2 * idx + 2, ho],
                        pt_[:, :, 1],
                        start=(idx == 0),
                        stop=(idx == 7),
                        perf_mode=DR,
                    )
                    if KDEBUG and pr == 0:
                        nc.sync.dma_start(dbg_pt[:, idx], pt_[:].bitcast(u8))

                # PV emission is delayed one chunk-pair: the PE queue is
                # strict FIFO for matmuls, so a PV waiting on its exp would
                # otherwise block the next (independent, ready) score MMs
                prev_pt = None
                for cc in range(8):
                    pt = ppool.tile([P, 2, 2, 512], f8, tag="pt")
                    for par in range(2):
                        c = 2 * cc + par
                        sc = smm.tile([P, 2, 512], f32, tag="smm")
                        nc.tensor.matmul(
                            sc[:, 0],
                            kT8[0:DH, pr, c * P : (c + 1) * P],
                            qT8[0:DH, pr],
                            start=True,
                            stop=True,
                        )
                        nc.tensor.matmul(
                            sc[:, 1],
                            kT8[DH:P, pr, c * P : (c + 1) * P],
                            qT8[DH:P, pr],
                            start=True,
                            stop=True,
                        )
                        if c in schr:
                            nc.vector.tensor_scalar(
                                pt[:, par].bitcast(u8),
                                sc[:],
                                A_SCHR,
                                B_SCHR,
                                OP.mult,
                                OP.add,
                            )
                        else:
                            nc.scalar.activation(pt[:, par], sc[:], AF.Exp, scale=SC_SCALE)
                        if fill:
                            fill.pop(0)()
                    if prev_pt is not None:
                        pv(cc - 1, prev_pt)
                    prev_pt = pt
                pv(7, prev_pt)
                if KDEBUG and pr == 0:
                    cpcopy = hpool.tile([P, 2, 512], f32, tag="cpdbg")
                    nc.vector.tensor_copy(cpcopy[:, 0], cps_e[:])
                    nc.vector.tensor_copy(cpcopy[:, 1], cps_o[:])
                    nc.sync.dma_start(dbg_cp[:], cpcopy[:])
                for st in fill:
                    st()
                if pr == 7 and KPHASE >= 3:
                    # four more j0-2 groups onto the smm banks (free once
                    # this pair's last exps have consumed them); they
                    # execute while the final normalize chain runs, keeping
                    # the HAM activity window alive through the tail
                    s1 = smm.tile([P, 2, 512], f32, tag="smm")
                    op_j02(2, 0, s1[:, 0])
                    op_j02(3, 0, s1[:, 1])
                    s2 = smm.tile([P, 2, 512], f32, tag="smm")
                    op_j02(0, 1, s2[:, 0])
                    op_j02(1, 1, s2[:, 1])
                # evacuate ctx PSUM to SBUF right away (frees the bank for the
                # next pair), then normalize from the SBUF copy
                for h, cps in ((he, cps_e), (ho, cps_o)):
                    rs = spool.tile([1, 512], f32, tag="rs")
                    nc.vector.tensor_copy(rs[:], cps[DH : DH + 1, :])
                    ri = spool.tile([1, 512], f32, tag="ri")
                    nc.vector.reciprocal_approx_fast(ri[:], rs[:])
                    rb = spool.tile([DH, 512], f32, tag="rb")
                    nc.gpsimd.partition_broadcast(rb[:], ri[:])
                    po = (h % 2) * DH
                    nc.vector.tensor_tensor(
                        ctxf[h // 4][po : po + DH, (h % 4) // 2],
                        cps[:DH],
                        rb[:],
                        OP.mult,
                    )

            # the last two j0-2 groups go to the ctxp banks, free once the
            # final pair's normalize has read them
            if KPHASE >= 3:
                c1 = ctxp.tile([P, 512], f32, tag="ctx")
                op_j02(2, 1, c1[:])
                c2 = ctxp.tile([P, 512], f32, tag="ctx")
                op_j02(3, 1, c2[:])

            # ---- out projection j3 + residual + LayerNorm ----
            for tt in range(4):
                if KPHASE < 2:
                    continue
                xr = xrs[tt]
                if KPHASE < 3:
                    nc.sync.dma_start(out_d[tt * P : (tt + 1) * P, :], xr[:])
                    continue
                h_sb = hpool.tile([P, D], f32, tag="h")
                for half in range(2):
                    ps = op_ps[(tt, half)]
                    # j3 needs the final pair's ctx; start=False keeps
                    # accumulating into the open j0-2 bank (has_written
                    # survives across groups)
                    nc.tensor.matmul(
                        ps,
                        ctxf[3][:, :, tt * P : (tt + 1) * P],
                        wo8_sb[:, 6:8, half * 512 : (half + 1) * 512],
                        start=False,
                        stop=True,
                        perf_mode=DR,
                        skip_group_check=True,
                    )
                    # residual (+bo folded into xres host-side, x512 scale)
                    nc.vector.tensor_tensor(
                        h_sb[:, half * 512 : (half + 1) * 512],
                        ps,
                        xr[:, half * 512 : (half + 1) * 512],
                        OP.add,
                    )
                if KPHASE == 4:
                    nc.sync.dma_start(out_d[tt * P : (tt + 1) * P, :], h_sb[:])
                    continue
                # LayerNorm over the free dim (scale-invariant; eps pre-scaled)
                s1 = spool.tile([P, 1], f32, tag="s1")
                nc.vector.reduce_sum(s1[:], h_sb[:], axis=AX)
                y = hpool.tile([P, D], f32, tag="y")
                s2 = spool.tile([P, 1], f32, tag="s2")
                nc.scalar.activation(y[:], h_sb[:], AF.Square, accum_out=s2[:])
                mu = spool.tile([P, 1], f32, tag="mu")
                nc.scalar.mul(mu[:], s1[:], 1.0 / D)
                m2 = spool.tile([P, 1], f32, tag="m2")
                nc.scalar.square(m2[:], mu[:])
                var = spool.tile([P, 1], f32, tag="var")
                nc.vector.tensor_scalar(
                    var[:], s2[:], 1.0 / D, m2[:], OP.mult, OP.subtract
                )
                sd = spool.tile([P, 1], f32, tag="sd")
                nc.scalar.activation(sd[:], var[:], AF.Sqrt, bias=eps_c[:], scale=1.0)
                rstd = spool.tile([P, 1], f32, tag="rstd")
                nc.vector.reciprocal(rstd[:], sd[:])
                nc.vector.tensor_scalar(
                    y[:], h_sb[:], mu[:], rstd[:], OP.subtract, OP.mult
                )
                if apply_gb:
                    nc.vector.tensor_tensor(y[:], y[:], ga_bc[:], OP.mult)
                    nc.vector.tensor_tensor(y[:], y[:], be_bc[:], OP.add)
                nc.sync.dma_start(out_d[tt * P : (tt + 1) * P, :512], y[:, :512])
                nc.sync.dma_start(out_d[tt * P : (tt + 1) * P, 512:], y[:, 512:])

            if KDEBUG:
                nc.sync.dma_start(dbg_k[:], kT8[:].bitcast(u8).rearrange("p a b -> p (a b)"))
                nc.sync.dma_start(dbg_q[:], qT8[:].bitcast(u8).rearrange("p a b -> p (a b)"))
                nc.sync.dma_start(dbg_v[:], v_sb[:].bitcast(u8).rearrange("p a b c -> p (a b c)"))
                for q_ in range(4):
                    nc.sync.dma_start(
                        dbg_c[:, q_ * 2 * TQ : (q_ + 1) * 2 * TQ],
                        ctxf[q_][:].bitcast(u8).rearrange("p a b -> p (a b)"),
                    )

    nc.compile()
    return nc


def _get_nc(apply_gb=True):
    key = ("nc", apply_gb)
    if key not in _BUILT:
        _BUILT[key] = _build_nc(apply_gb)
    return _BUILT[key]


def _prep_in_maps(x, Wq, bq, Wk, bk, Wv, bv, Wo, bo, gamma, beta):
    x = np.asarray(x, F32)
    wq = np.ascontiguousarray(WQ_S * np.asarray(Wq, F32).T).astype(F8)
    wk = np.ascontiguousarray(WK_S * np.asarray(Wk, F32).T).astype(F8)
    wv = np.ascontiguousarray(WV_S * np.asarray(Wv, F32).T).astype(F8)
    wo = np.ascontiguousarray(WO_S * np.asarray(Wo, F32).T).astype(F8)
    qb = np.ascontiguousarray(WQ_S * np.asarray(bq, F32).reshape(8, P).T)
    kb = np.ascontiguousarray(WK_S * np.asarray(bk, F32).reshape(8, P).T)
    rows = (
        np.concatenate(
            [WV_S * np.asarray(bv, F32), np.asarray(gamma, F32), np.asarray(beta, F32)]
        )
        .reshape(1, 3 * D)
        .astype(BF16)
    )
    bo = np.asarray(bo, F32)
    xT = [np.ascontiguousarray(x[b].T) for b in range(B)]

    in_maps = []
    for c in range(N_CORES):
        b, q = c // 4, c % 4
        # permute: own query strip first; key order is irrelevant to attention
        perm = np.r_[q * TQ : (q + 1) * TQ, 0 : q * TQ, (q + 1) * TQ : S]
        in_maps.append(
            {
                # strip-major [strip, p, k, 512] (see kernel DMA order)
                "x8": np.ascontiguousarray(
                    xT[b][:, perm].reshape(8, P, 4, 512).transpose(2, 1, 0, 3)
                ).astype(F8),
                "wq": wq,
                "wk": wk,
                "wv": wv,
                "wo": wo,
                "qb": qb,
                "kb": kb,
                "rows": rows,
                "xres": RES_S
                * (np.ascontiguousarray(x[b, q * TQ : (q + 1) * TQ, :]) + bo[None, :]),
            }
        )
    return in_maps


def kernel(x, Wq, bq, Wk, bk, Wv, bv, Wo, bo, gamma, beta):
    from concourse.bass_utils import run_bass_kernel_spmd

    apply_gb = not (
        np.all(np.asarray(gamma, F32) == 1.0) and np.all(np.asarray(beta, F32) == 0.0)
    )
    nc = _get_nc(apply_gb)
    in_maps = _prep_in_maps(x, Wq, bq, Wk, bk, Wv, bv, Wo, bo, gamma, beta)
    res = run_bass_kernel_spmd(nc, in_maps, core_ids=list(range(N_CORES)))
    out = np.empty((B, S, D), F32)
    for c in range(N_CORES):
        b, q = c // 4, c % 4
        out[b, q * TQ : (q + 1) * TQ, :] = res.results[c]["out"]
    return out



# revision 18
# speedup vs baseline: 1.0500x; 1.0500x over previous
"""Trainium2 Bass kernel for MultiHeadSelfAttention + residual + LayerNorm.

Problem: x[2, 2048, 1024], 16 heads, head_dim 64, fp32 I/O.
  Q/K/V = x @ W{q,k,v}.T + b;  attn = softmax(Q K^T / 8) V
  out = attn-concat @ Wo.T + bo;  y = LayerNorm(x + out)

Sharding (8 cores, collective-free):
  core c: batch b = c // 4, query-token strip q = c % 4 (512 tokens).
  Each core computes K/V for its whole batch (all 16 heads), Q for its
  512 query tokens, full attention + out-proj + LayerNorm for them, and
  outputs out[512, 1024].  K/V projection is recomputed 4x per batch --
  cheaper than the measured collective alternatives for this shape.

v2 design (PE was the bottleneck at ~259us busy of a 310us span):
  - ALL matmuls run fp8 e4m3 (x, Wq, Wk, Wv, Wo, K, Q, P, V, ctx in fp8)
    with DoubleRow pair-contraction for the projections and out-proj:
    halves the MM count of K/Q/out projections vs the bf16 baseline.
  - Weights are pre-scaled host-side to sit in the e4m3 normal range
    (Wq,Wk x8; Wv x16; Wo x32) and the residual input is pre-scaled x512
    so the out-proj PSUM lands at 512*(out+x).  LayerNorm is
    scale-invariant, so only eps is scaled (x512^2); gamma/beta epilogue
    is unaffected.  Score scale absorbs the 8*8: exp(scale=0.125/64).
  - Scores (contraction = head_dim 64 -> only half the PE rows) run as
    row-tiled CONCURRENT pairs: head-even weights in array rows 0-63,
    head-odd in rows 64-127 (tile_position auto-derived from the base
    partition), sharing the 128x128 array per key chunk.
  - x is loaded once (fp8, 2MB) and stays resident; the bf16 copy of x
    is gone entirely (halves input DMA).
  - softmax exp: most chunks on the Scalar engine (Exp LUT, fp8 out);
    every 4th chunk is computed on the Vector engine instead with a
    Schraudolph bit-trick: bits = round(a*logits + 55.54) as uint8,
    bitcast to e4m3 (DVE converts with round-to-nearest, saturating).
    Zero-bias constant so ACT and DVE chunks agree in expectation;
    softmax renormalizes the shared multiplicative bias away anyway
    (the ones-column in V gives rowsums of the SAME quantized P).
  - rowsum reciprocal runs directly on the PSUM rowsum row ([1,512]),
    then gpsimd partition-broadcasts the reciprocal (drops one DVE
    copy per head vs broadcasting the raw sum).
  - Only K/Q j-tiles 0-1 run before the attention loop; V quads 0..3 and
    the remaining j-tiles stream in as PE filler inside the attention
    windows (scheduled so each pair's K/Q/V inputs land one pair ahead),
    so the first exp fires ~15us in, the PE never idles long, and the
    HAM clock stays warm.
  - rowsum reciprocal: DVE copy of the PSUM rowsum row -> fp32
    reciprocal_approx_fast -> gpsimd partition-broadcast.  (The custom
    DVE reciprocal silently corrupts when its input AP has a nonzero
    base partition or lives in PSUM -- keep it fed from an SBUF tile at
    partition 0.)
  - The upfront j0/j1 K/Q evacuations run on the Scalar engine (Identity
    + per-partition bias) in its pre-exp idle window; out-proj PSUM for
    both halves comes from the pmm pool (idle after the last proj
    filler) so its j<3 accumulation MMs hoist under pairs 6-7 instead of
    waiting for the final exp to free an smm bank.  wk8 is DMA'd in
    column halves (j0-3 first) and the last pair's final chunk runs on
    the DVE so the closing PV never queues behind ACT's FIFO.
  - PV emission is delayed one chunk-pair: the PE executes matmuls in
    strict FIFO order, so a PV waiting on its exp output would block the
    next chunk's (independent, already-ready) score matmuls; delaying PV
    one iteration fills that bubble with the following scores.
  - V projection runs as token-chunk x 8-head (N=512) matmul groups:
    half the matmul count and half the DVE evacuations of the older
    4-head/N=256 quads.
  - PSUM (8 banks) is the binding constraint on further restructuring:
    score double-buffering (4) + two ctx accumulators (2) + proj-filler
    double-buffering (2) is exactly 8, which rules out 4-head row-tiled
    scores and 2048-wide exp batches (both need >=10 banks).
Measured: HW exec ~228-230us (baseline bf16 kernel: ~300us); end-to-end
Frobenius rel err ~6.7e-4 (tolerance 2e-2; errors in the attention path
are suppressed ~100x by the residual, so fp8 everywhere is safe).
Note: cross-core AllGather K/V sharing was implemented and was
numerically correct, but each 0.5MB collective costs ~20us serial
latency under this runtime and two collectives serialize on gpsimd --
it measured 323us and was reverted.
"""

import numpy as np
import ml_dtypes

P = 128
D = 1024
S = 2048
B = 2
H = 16
DH = 64
TQ = 512  # query tokens per core
N_CORES = 8

F32 = np.float32
BF16 = ml_dtypes.bfloat16
F8 = ml_dtypes.float8_e4m3fn

# host-side pre-scales (see docstring)
WQ_S = 8.0
WK_S = 8.0
WV_S = 16.0
WO_S = 32.0
RES_S = WV_S * WO_S  # 512
LN_EPS = 1e-5 * RES_S * RES_S
SC_SCALE = 0.125 / (WQ_S * WK_S)  # exp scale on raw psum logits
A_SCHR = SC_SCALE * 8.0 / 0.6931471805599453
B_SCHR = 55.54
# chunk indices (of 16 per head-pair) done on DVE instead of ACT.
# Pairs 0-3 are PE-bound (projection fillers) so ACT takes most chunks.
# Pairs 4-7 are bound by the exp->score smm-bank recycle chain: with the
# 2-buf rotation, chunk c's scores wait on chunk c-2's exp, so EVEN and
# ODD chunks form two independent chains.  Keeping all DVE chunks at odd
# indices dedicates the even chain to ACT and (partially) the odd chain
# to DVE, letting the two chains advance concurrently.
SCHR_BY_PAIR = {
    0: (3, 8, 13),
    1: (3, 8, 13),
    2: (3, 8, 13),
    3: (3, 8, 13),
    4: (3, 7, 11, 15),
    5: (3, 7, 11, 15),
    6: (1, 3, 5, 7, 9, 11, 13, 15),
    7: (1, 3, 5, 7, 9, 11, 13, 15),
}

_BUILT = {}

import os

KPHASE = int(os.environ.get("KPHASE", "3"))
KDEBUG = int(os.environ.get("KDEBUG", "0"))


def _build_nc(apply_gb=True):
    from contextlib import ExitStack

    import concourse.tile as tile
    from concourse import bacc, mybir

    bf = mybir.dt.bfloat16
    f8 = mybir.dt.float8e4
    u8 = mybir.dt.uint8
    f32 = mybir.dt.float32
    AX = mybir.AxisListType.X
    OP = mybir.AluOpType
    AF = mybir.ActivationFunctionType
    DR = mybir.MatmulPerfMode.DoubleRow

    nc = bacc.Bacc(
        "TRN2",
        target_bir_lowering=False,
        debug=False,
        enable_asserts=False,
        num_devices=N_CORES,
    )

    # ---- DRAM I/O ----
    x8_d = nc.dram_tensor("x8", [8, P, S], f8, kind="ExternalInput").ap()
    wq_d = nc.dram_tensor("wq", [D, D], f8, kind="ExternalInput").ap()
    wk_d = nc.dram_tensor("wk", [D, D], f8, kind="ExternalInput").ap()
    wv_d = nc.dram_tensor("wv", [D, D], f8, kind="ExternalInput").ap()
    wo_d = nc.dram_tensor("wo", [D, D], f8, kind="ExternalInput").ap()
    qb_d = nc.dram_tensor("qb", [P, 8], f32, kind="ExternalInput").ap()
    kb_d = nc.dram_tensor("kb", [P, 8], f32, kind="ExternalInput").ap()
    # rows: [16*bv | gamma | beta]
    rows_d = nc.dram_tensor("rows", [1, 3 * D], bf, kind="ExternalInput").ap()
    xres_d = nc.dram_tensor("xres", [TQ, D], f32, kind="ExternalInput").ap()
    out_d = nc.dram_tensor("out", [TQ, D], f32, kind="ExternalOutput").ap()
    if KDEBUG:
        dbg_k = nc.dram_tensor("dbg_k", [P, 8 * S], u8, kind="ExternalOutput").ap()
        dbg_q = nc.dram_tensor("dbg_q", [P, 8 * TQ], u8, kind="ExternalOutput").ap()
        dbg_v = nc.dram_tensor("dbg_v", [P, 16 * H * (DH + 1)], u8, kind="ExternalOutput").ap()
        dbg_c = nc.dram_tensor("dbg_c", [P, 8 * TQ], u8, kind="ExternalOutput").ap()
        dbg_pt = nc.dram_tensor("dbg_pt", [P, 8, 2, 2, 512], u8, kind="ExternalOutput").ap()
        dbg_cp = nc.dram_tensor("dbg_cp", [P, 2, 512], f32, kind="ExternalOutput").ap()

    wq_t = wq_d.rearrange("(o p) n -> p o n", p=P)  # [128, 8, 1024]
    wk_t = wk_d.rearrange("(o p) n -> p o n", p=P)
    wv_t = wv_d.rearrange("(o p) n -> p o n", p=P)
    wo_t = wo_d.rearrange("(o p) n -> p o n", p=P)

    with tile.TileContext(nc) as tc:
        with ExitStack() as ctx:
            # ---- pools ----
            consts = ctx.enter_context(tc.tile_pool(name="consts", bufs=1))
            wpool = ctx.enter_context(tc.tile_pool(name="wpool", bufs=1))
            big = ctx.enter_context(tc.tile_pool(name="big", bufs=1))
            ppool = ctx.enter_context(tc.tile_pool(name="ppool", bufs=3))
            spool = ctx.enter_context(tc.tile_pool(name="spool", bufs=4))
            hpool = ctx.enter_context(tc.tile_pool(name="hpool", bufs=3))
            xrpool = ctx.enter_context(tc.tile_pool(name="xrpool", bufs=4))
            pmm = ctx.enter_context(tc.tile_pool(name="pmm", bufs=2, space="PSUM"))
            smm = ctx.enter_context(tc.tile_pool(name="smm", bufs=2, space="PSUM"))
            ctxp = ctx.enter_context(tc.tile_pool(name="ctxp", bufs=2, space="PSUM"))

            # ---- constants ----
            zero_c = consts.tile([P, 1], f32, tag="zero_c")
            nc.vector.memset(zero_c[:], 0.0)
            nc.const_aps.aps[(f32, 0.0)] = zero_c[:]
            eps_c = consts.tile([P, 1], f32, tag="eps_c")
            nc.vector.memset(eps_c[:], LN_EPS)
            ones_l = consts.tile([1, P], bf, tag="ones_l")  # matmul lhsT ones
            nc.vector.memset(ones_l[:], 1.0)
            rows_sb = consts.tile([1, 3 * D], bf, tag="rows")
            nc.sync.dma_start(rows_sb[:], rows_d[:])
            qb_sb = consts.tile([P, 8], f32, tag="qb")
            nc.sync.dma_start(qb_sb[:], qb_d[:])
            kb_sb = consts.tile([P, 8], f32, tag="kb")
            nc.sync.dma_start(kb_sb[:], kb_d[:])

            # ---- resident inputs ----
            x8_sb = wpool.tile([P, 8, S], f8, tag="x8")
            wk8_sb = wpool.tile([P, 8, D], f8, tag="wk8")
            wv8_sb = wpool.tile([P, 8, D], f8, tag="wv8")
            wq8_sb = wpool.tile([P, 8, D], f8, tag="wq8")
            wo8_sb = wpool.tile([P, 8, D], f8, tag="wo8")
            # DMA waves matched to compute order: wave 1 (x + wk half0)
            # gates the upfront K projections; wq-h0 next (Q j0/j1 + the
            # j2/j3 fillers), then wv-h0 (pair-0 V fillers); the remaining
            # column halves are not needed until pair 2+ (~60us) and come
            # last.
            for k in range(8):
                nc.sync.dma_start(x8_sb[:, k], x8_d[k])
                nc.sync.dma_start(wk8_sb[:, k, :512], wk_t[:, k, :512])
            for k in range(8):
                nc.sync.dma_start(wq8_sb[:, k, :512], wq_t[:, k, :512])
            for k in range(8):
                nc.sync.dma_start(wv8_sb[:, k, :512], wv_t[:, k, :512])
            for k in range(8):
                nc.sync.dma_start(wk8_sb[:, k, 512:], wk_t[:, k, 512:])
            for k in range(8):
                nc.sync.dma_start(wq8_sb[:, k, 512:], wq_t[:, k, 512:])
            for k in range(8):
                nc.sync.dma_start(wv8_sb[:, k, 512:], wv_t[:, k, 512:])

            # broadcast [1, 1024] rows across partitions via rank-1 matmuls
            bv_bc = consts.tile([P, D], bf, tag="bv_bc")
            bcasts = [bv_bc]
            if apply_gb:
                ga_bc = consts.tile([P, D], bf, tag="ga_bc")
                be_bc = consts.tile([P, D], bf, tag="be_bc")
                bcasts += [ga_bc, be_bc]
            for idx, dst in enumerate(bcasts):
                for half in range(2):
                    ps = smm.tile([P, 2, 512], f32, tag="smm")
                    nc.tensor.matmul(
                        ps[:, 0],
                        ones_l[:],
                        rows_sb[:, idx * D + half * 512 : idx * D + (half + 1) * 512],
                        start=True,
                        stop=True,
                    )
                    nc.scalar.copy(dst[:, half * 512 : (half + 1) * 512], ps[:, 0])

            # ---- big activations ----
            kT8 = big.tile([P, 8, S], f8, tag="kT")  # K^T: [dh-pair part, j, token]
            qT8 = big.tile([P, 8, TQ], f8, tag="qT")
            # V' per (tk-chunk, head): [128 tok, 65] (64 dh + ones col)
            v_sb = big.tile([P, 16, H, DH + 1], f8, tag="v")
            nc.vector.memset(v_sb[:, :, :, DH : DH + 1], 1.0)
            ctxf = [
                big.tile([P, 2, TQ], f8, tag=f"ctxf{q}", name=f"ctxf{q}")
                for q in range(4)
            ]

            # ---- fp8 DoubleRow K/Q projection for one j-tile (all strips) ----
            # evac_act=True moves the bias-add evacuation to the Scalar
            # engine (Identity + per-partition bias) -- used for the
            # upfront j-tiles, which land in ACT's pre-exp idle window
            def kq_proj_j(j, evac_act=False):
                def evac(dst, ps, bias):
                    if evac_act:
                        nc.scalar.activation(dst, ps, AF.Identity, bias=bias)
                    else:
                        nc.vector.tensor_scalar_add(dst, ps, bias)

                steps = []
                for s in range(4):
                    def kstep(s=s, j=j):
                        ps = pmm.tile([P, 512], f32, tag="pmm")
                        for c2 in range(4):
                            nc.tensor.matmul(
                                ps[:],
                                wk8_sb[:, 2 * c2 : 2 * c2 + 2, j * P : (j + 1) * P],
                                x8_sb[:, 2 * c2 : 2 * c2 + 2, s * 512 : (s + 1) * 512],
                                start=(c2 == 0),
                                stop=(c2 == 3),
                                perf_mode=DR,
                            )
                        evac(
                            kT8[:, j, s * 512 : (s + 1) * 512], ps[:], kb_sb[:, j : j + 1]
                        )
                    steps.append(kstep)

                def qstep(j=j):
                    ps = pmm.tile([P, 512], f32, tag="pmm")
                    for c2 in range(4):
                        nc.tensor.matmul(
                            ps[:],
                            wq8_sb[:, 2 * c2 : 2 * c2 + 2, j * P : (j + 1) * P],
                            x8_sb[:, 2 * c2 : 2 * c2 + 2, 0:512],
                            start=(c2 == 0),
                            stop=(c2 == 3),
                            perf_mode=DR,
                        )
                    evac(qT8[:, j], ps[:], qb_sb[:, j : j + 1])
                steps.append(qstep)
                return steps

            # ---- fp8 DoubleRow V projection: one token chunk x 8 heads
            # (N=512) per step -- half the MM and evacuation count of the
            # old 4-head/N=256 quads ----
            def v_proj_half(half):
                steps = []
                for tchunk in range(16):
                    def vstep(tchunk=tchunk, half=half):
                        ps = pmm.tile([P, 512], f32, tag="pmm")
                        for c2 in range(4):
                            nc.tensor.matmul(
                                ps[:],
                                x8_sb[:, 2 * c2 : 2 * c2 + 2, tchunk * P : (tchunk + 1) * P],
                                wv8_sb[:, 2 * c2 : 2 * c2 + 2, half * 512 : (half + 1) * 512],
                                start=(c2 == 0),
                                stop=(c2 == 3),
                                perf_mode=DR,
                            )
                        nc.vector.tensor_tensor(
                            v_sb[:, tchunk, half * 8 : (half + 1) * 8, 0:DH],
                            ps[:].rearrange("p (h d) -> p h d", d=DH),
                            bv_bc[:, half * 512 : (half + 1) * 512].rearrange(
                                "p (h d) -> p h d", d=DH
                            ),
                            OP.add,
                        )
                    steps.append(vstep)
                return steps

            # upfront: j0, j1 K/Q projections only; V half 0 streams in as
            # pair-0 filler.  Q j0 is emitted right after the K j0 strips
            # (wq half0 is DMA wave 2, resident by then) so pair 0's first
            # score matmuls are unblocked as early as possible.
            up0 = kq_proj_j(0, evac_act=True)
            up1 = kq_proj_j(1, evac_act=True)
            for st in up0[:4]:
                st()
            up0[4]()
            for st in up1[:4]:
                st()
            up1[4]()

            # filler schedule per attention pair (consumed one per chunk)
            def dma_fill():
                for k in range(8):
                    nc.sync.dma_start(wo8_sb[:, k], wo_t[:, k])

            xrs = []

            def xres_fill():
                for tt in range(4):
                    xr = xrpool.tile([P, D], f32, tag="xr", name=f"xr{tt}")
                    nc.sync.dma_start(xr[:], xres_d[tt * P : (tt + 1) * P, :])
                    xrs.append(xr)

            vh = [v_proj_half(h) for h in range(2)]

            # ---- out-projection j0-2 accumulation groups ----
            # Emitted early (pair-6 fillers + right after pair 7's last PV)
            # so they execute during the otherwise PE-idle exp/normalize
            # windows of the last two pairs; all 8 PSUM banks carry an open
            # out-proj group at the tail.  j3 (which needs the final pair's
            # ctx) lands in the same banks via start=False accumulation.
            op_ps = {}

            def op_j02(tt, half, ps):
                op_ps[(tt, half)] = ps
                for j in range(3):
                    nc.tensor.matmul(
                        ps,
                        ctxf[j][:, :, tt * P : (tt + 1) * P],
                        wo8_sb[:, 2 * j : 2 * j + 2, half * 512 : (half + 1) * 512],
                        start=(j == 0),
                        stop=(j == 2),
                        perf_mode=DR,
                    )

            def op_fill(tt, half):
                def step():
                    ps = pmm.tile([P, 512], f32, tag="pmm")
                    op_j02(tt, half, ps[:])
                return step

            # wo8/xres prefetch sits at pair 3, NOT the tail: the out-proj
            # j0-2 groups can only hoist under pairs 6-7 if wo8 has landed
            fillers = [
                vh[0],
                kq_proj_j(2) + kq_proj_j(3) + vh[1][:6],
                vh[1][6:] + kq_proj_j(4),
                kq_proj_j(5) + [dma_fill, xres_fill],
                kq_proj_j(6),
                kq_proj_j(7),
                [op_fill(0, 0), op_fill(1, 0)] if KPHASE >= 3 else [],
                [],
            ]

            # ---- attention: 8 head-pairs, row-tiled concurrent score MMs ----
            for pr in range(8):
                he, ho = 2 * pr, 2 * pr + 1
                # pairs 4-7 end on a DVE chunk so the closing PV+normalize
                # chain never queues behind ACT's FIFO
                schr = SCHR_BY_PAIR[pr]
                fill = list(fillers[pr])
                if KPHASE < 2:
                    for st in fill:
                        st()
                    continue
                cps_e = ctxp.tile([P, 512], f32, tag="ctx")
                cps_o = ctxp.tile([P, 512], f32, tag="ctx")

                # PV for chunk-pair idx (both heads)
                def pv(idx, pt_):
                    nc.tensor.matmul(
                        cps_e[: DH + 1],
                        v_sb[:, 2 * idx : 2 * idx + 2, he],
                        pt_[:, :, 0],
                        start=(idx == 0),
                        stop=(idx == 7),
                        perf_mode=DR,
                    )
                    nc.tensor.matmul(
                        cps_o[: DH + 1],
                        v_sb[:, 2 * idx : 2 * idx + 2, ho],
                        pt_[:, :, 1],
                        start=(idx == 0),
                        stop=(idx == 7),
                        perf_mode=DR,
                    )
                    if KDEBUG and pr == 0:
                        nc.sync.dma_start(dbg_pt[:, idx], pt_[:].bitcast(u8))

                # PV emission is delayed one chunk-pair: the PE queue is
                # strict FIFO for matmuls, so a PV waiting on its exp would
                # otherwise block the next (independent, ready) score MMs
                prev_pt = None
                for cc in range(8):
                    pt = ppool.tile([P, 2, 2, 512], f8, tag="pt")
                    for par in range(2):
                        c = 2 * cc + par
                        sc = smm.tile([P, 2, 512], f32, tag="smm")
                        nc.tensor.matmul(
                            sc[:, 0],
                            kT8[0:DH, pr, c * P : (c + 1) * P],
                            qT8[0:DH, pr],
                            start=True,
                            stop=True,
                        )
                        nc.tensor.matmul(
                            sc[:, 1],
                            kT8[DH:P, pr, c * P : (c + 1) * P],
                            qT8[DH:P, pr],
                            start=True,
                            stop=True,
                        )
                        if c in schr:
                            nc.vector.tensor_scalar(
                                pt[:, par].bitcast(u8),
                                sc[:],
                                A_SCHR,
                                B_SCHR,
                                OP.mult,
                                OP.add,
                            )
                        else:
                            nc.scalar.activation(pt[:, par], sc[:], AF.Exp, scale=SC_SCALE)
                        if fill:
                            fill.pop(0)()
                    if prev_pt is not None:
                        pv(cc - 1, prev_pt)
                    prev_pt = pt
                pv(7, prev_pt)
                if KDEBUG and pr == 0:
                    cpcopy = hpool.tile([P, 2, 512], f32, tag="cpdbg")
                    nc.vector.tensor_copy(cpcopy[:, 0], cps_e[:])
                    nc.vector.tensor_copy(cpcopy[:, 1], cps_o[:])
                    nc.sync.dma_start(dbg_cp[:], cpcopy[:])
                for st in fill:
                    st()
                if pr == 7 and KPHASE >= 3:
                    # four more j0-2 groups onto the smm banks (free once
                    # this pair's last exps have consumed them); they
                    # execute while the final normalize chain runs, keeping
                    # the HAM activity window alive through the tail
                    s1 = smm.tile([P, 2, 512], f32, tag="smm")
                    op_j02(2, 0, s1[:, 0])
                    op_j02(3, 0, s1[:, 1])
                    s2 = smm.tile([P, 2, 512], f32, tag="smm")
                    op_j02(0, 1, s2[:, 0])
                    op_j02(1, 1, s2[:, 1])
                # evacuate ctx PSUM to SBUF right away (frees the bank for the
                # next pair), then normalize from the SBUF copy
                for h, cps in ((he, cps_e), (ho, cps_o)):
                    rs = spool.tile([1, 512], f32, tag="rs")
                    nc.vector.tensor_copy(rs[:], cps[DH : DH + 1, :])
                    ri = spool.tile([1, 512], f32, tag="ri")
                    nc.vector.reciprocal_approx_fast(ri[:], rs[:])
                    rb = spool.tile([DH, 512], f32, tag="rb")
                    nc.gpsimd.partition_broadcast(rb[:], ri[:])
                    po = (h % 2) * DH
                    nc.vector.tensor_tensor(
                        ctxf[h // 4][po : po + DH, (h % 4) // 2],
                        cps[:DH],
                        rb[:],
                        OP.mult,
                    )

            # the last two j0-2 groups go to the ctxp banks, free once the
            # final pair's normalize has read them
            if KPHASE >= 3:
                c1 = ctxp.tile([P, 512], f32, tag="ctx")
                op_j02(2, 1, c1[:])
                c2 = ctxp.tile([P, 512], f32, tag="ctx")
                op_j02(3, 1, c2[:])

            # ---- out projection j3 + residual + LayerNorm ----
            for tt in range(4):
                if KPHASE < 2:
                    continue
                xr = xrs[tt]
                if KPHASE < 3:
                    nc.sync.dma_start(out_d[tt * P : (tt + 1) * P, :], xr[:])
                    continue
                h_sb = hpool.tile([P, D], f32, tag="h")
                for half in range(2):
                    ps = op_ps[(tt, half)]
                    # j3 needs the final pair's ctx; start=False keeps
                    # accumulating into the open j0-2 bank (has_written
                    # survives across groups)
                    nc.tensor.matmul(
                        ps,
                        ctxf[3][:, :, tt * P : (tt + 1) * P],
                        wo8_sb[:, 6:8, half * 512 : (half + 1) * 512],
                        start=False,
                        stop=True,
                        perf_mode=DR,
                        skip_group_check=True,
                    )
                    # residual (+bo folded into xres host-side, x512 scale)
                    nc.vector.tensor_tensor(
                        h_sb[:, half * 512 : (half + 1) * 512],
                        ps,
                        xr[:, half * 512 : (half + 1) * 512],
                        OP.add,
                    )
                if KPHASE == 4:
                    nc.sync.dma_start(out_d[tt * P : (tt + 1) * P, :], h_sb[:])
                    continue
                # LayerNorm over the free dim (scale-invariant; eps pre-scaled)
                s1 = spool.tile([P, 1], f32, tag="s1")
                nc.vector.reduce_sum(s1[:], h_sb[:], axis=AX)
                y = hpool.tile([P, D], f32, tag="y")
                s2 = spool.tile([P, 1], f32, tag="s2")
                nc.scalar.activation(y[:], h_sb[:], AF.Square, accum_out=s2[:])
                mu = spool.tile([P, 1], f32, tag="mu")
                nc.scalar.mul(mu[:], s1[:], 1.0 / D)
                m2 = spool.tile([P, 1], f32, tag="m2")
                nc.scalar.square(m2[:], mu[:])
                var = spool.tile([P, 1], f32, tag="var")
                nc.vector.tensor_scalar(
                    var[:], s2[:], 1.0 / D, m2[:], OP.mult, OP.subtract
                )
                sd = spool.tile([P, 1], f32, tag="sd")
                nc.scalar.activation(sd[:], var[:], AF.Sqrt, bias=eps_c[:], scale=1.0)
                rstd = spool.tile([P, 1], f32, tag="rstd")
                nc.vector.reciprocal(rstd[:], sd[:])
                nc.vector.tensor_scalar(
                    y[:], h_sb[:], mu[:], rstd[:], OP.subtract, OP.mult
                )
                if apply_gb:
                    nc.vector.tensor_tensor(y[:], y[:], ga_bc[:], OP.mult)
                    nc.vector.tensor_tensor(y[:], y[:], be_bc[:], OP.add)
                nc.sync.dma_start(out_d[tt * P : (tt + 1) * P, :512], y[:, :512])
                nc.sync.dma_start(out_d[tt * P : (tt + 1) * P, 512:], y[:, 512:])

            if KDEBUG:
                nc.sync.dma_start(dbg_k[:], kT8[:].bitcast(u8).rearrange("p a b -> p (a b)"))
                nc.sync.dma_start(dbg_q[:], qT8[:].bitcast(u8).rearrange("p a b -> p (a b)"))
                nc.sync.dma_start(dbg_v[:], v_sb[:].bitcast(u8).rearrange("p a b c -> p (a b c)"))
                for q_ in range(4):
                    nc.sync.dma_start(
                        dbg_c[:, q_ * 2 * TQ : (q_ + 1) * 2 * TQ],
                        ctxf[q_][:].bitcast(u8).rearrange("p a b -> p (a b)"),
                    )

    nc.compile()
    return nc


def _get_nc(apply_gb=True):
    key = ("nc", apply_gb)
    if key not in _BUILT:
        _BUILT[key] = _build_nc(apply_gb)
    return _BUILT[key]


def _prep_in_maps(x, Wq, bq, Wk, bk, Wv, bv, Wo, bo, gamma, beta):
    x = np.asarray(x, F32)
    wq = np.ascontiguousarray(WQ_S * np.asarray(Wq, F32).T).astype(F8)
    wk = np.ascontiguousarray(WK_S * np.asarray(Wk, F32).T).astype(F8)
    wv = np.ascontiguousarray(WV_S * np.asarray(Wv, F32).T).astype(F8)
    wo = np.ascontiguousarray(WO_S * np.asarray(Wo, F32).T).astype(F8)
    qb = np.ascontiguousarray(WQ_S * np.asarray(bq, F32).reshape(8, P).T)
    kb = np.ascontiguousarray(WK_S * np.asarray(bk, F32).reshape(8, P).T)
    rows = (
        np.concatenate(
            [WV_S * np.asarray(bv, F32), np.asarray(gamma, F32), np.asarray(beta, F32)]
        )
        .reshape(1, 3 * D)
        .astype(BF16)
    )
    bo = np.asarray(bo, F32)
    xT = [np.ascontiguousarray(x[b].T) for b in range(B)]

    in_maps = []
    for c in range(N_CORES):
        b, q = c // 4, c % 4
        # permute: own query strip first; key order is irrelevant to attention
        perm = np.r_[q * TQ : (q + 1) * TQ, 0 : q * TQ, (q + 1) * TQ : S]
        in_maps.append(
            {
                "x8": np.ascontiguousarray(
                    xT[b][:, perm].reshape(8, P, S)
                ).astype(F8),
                "wq": wq,
                "wk": wk,
                "wv": wv,
                "wo": wo,
                "qb": qb,
                "kb": kb,
                "rows": rows,
                "xres": RES_S
                * (np.ascontiguousarray(x[b, q * TQ : (q + 1) * TQ, :]) + bo[None, :]),
            }
        )
    return in_maps


def kernel(x, Wq, bq, Wk, bk, Wv, bv, Wo, bo, gamma, beta):
    from concourse.bass_utils import run_bass_kernel_spmd

    apply_gb = not (
        np.all(np.asarray(gamma, F32) == 1.0) and np.all(np.asarray(beta, F32) == 0.0)
    )
    nc = _get_nc(apply_gb)
    in_maps = _prep_in_maps(x, Wq, bq, Wk, bk, Wv, bv, Wo, bo, gamma, beta)
    res = run_bass_kernel_spmd(nc, in_maps, core_ids=list(range(N_CORES)))
    out = np.empty((B, S, D), F32)
    for c in range(N_CORES):
        b, q = c // 4, c % 4
        out[b, q * TQ : (q + 1) * TQ, :] = res.results[c]["out"]
    return out

